# revision 1
# baseline (speedup 1.0000x reference)
"""Trainium2 Bass kernel for BatchRankingLoss.

Reference computation (B=131072, d=256 decoys, K=512 complexes, G=K-1=511 groups):
    o, t -> reshape to [G, d]
    dt = t_i - t_j ; y = sign-ish(dt) ; w = |dt| > 0.1
    dL = w * max(0, 1 + y*(o_i - o_j)) ; loss = sum(dL) / (G*d*(d-1))

Key identity used on device: dL is symmetric in (i,j) for |dt|>0.1 pairs, so
    sum(dL) = 2 * sum_{(i,j): dt_ij > 0.1} relu(1 + o_i - o_j)

Device computation per core (64 groups/core, group axis sharded over 8 cores):
  partition p in [0,128) = (g_local = p//2, half = p%2); per partition the free
  axis enumerates (i_local in [0,128), j in [0,256)) = 32768 elements, walked in
  32 chunks of [128, 1024] (each chunk = two N=512 matmul slices).

  - PE:  u   = (t_i - 0.1) - t_j  via K=66 block-diagonal float32r matmul -> PSUM
         do1 = (1 + o_i) - o_j    via K=66 block-diagonal float32r matmul -> PSUM
         (weights [66,128] per slice: rows 0-63 = group-indicator block, rows
          64-65 = t'/o' values for the slice's two i_local values; moving
          operand fixed [66,512]: rows 0-63 = -t_j/-o_j rows, 64-65 = i-slot
          indicators)
  - ACT: h = relu(do1)  (PSUM -> SBUF)
  - DVE: m = (u is_gt 0) * h  via scalar_tensor_tensor (exact mask),
         accum_out -> m_acc column
  Host:  loss = 2 * sum(m_acc over all cores) / N
"""

import numpy as np
from contextlib import ExitStack

import concourse.bacc as bacc
import concourse.mybir as mybir
import concourse.tile as tile
from concourse.bass_utils import run_bass_kernel_spmd

N_CORES = 8
D = 256                 # decoys per complex
G_REAL = 511            # torch loop skips the final group
G_PAD = 512             # pad with a zero group so every core gets 64
GPC = G_PAD // N_CORES  # 64 groups per core
P = 128                 # partitions = GPC * 2 halves
IPB = 128               # i_local values per partition (= D/2)
N_SLICES = 64           # 512-wide matmul slices per sweep (2 i_local each)
N_CHUNKS = 32           # [128, 1024] chunks (2 slices each)
KDIM = GPC + 2          # matmul contraction: 64 group rows + 2 i-slot rows
N_PAIRS = G_REAL * D * (D - 1)

# Groups are host-sorted by t, so pairs with dt > 0.1 all have j < i (global
# i = half*128 + i_local). Slice s (i_local = 2s, 2s+1) only needs columns
# j < 130 + 2s (the half=1 partition's bound); the rest are masked anyway.
import os
JMAX = ([D] * N_SLICES if os.environ.get("JMAX_FULL") == "1"
        else [min(D, 130 + 2 * s) for s in range(N_SLICES)])
EXT = [2 * j for j in JMAX]                       # free extent of slice s
CHUNK_EXT = [EXT[2 * c] + EXT[2 * c + 1] for c in range(N_CHUNKS)]

_CACHED = {}


def _build_program(repeat=1, mode="full", loop=0):
    """Build the SPMD program. repeat>1 re-runs the compute loop in-NEFF
    (identical work, same outputs) for wall-clock delta timing; loop>0 wraps
    the compute loop in a hardware For_i loop instead (low-noise timing, adds
    a ~2us all-engine barrier per iteration). mode
    ("mm"|"mm_act"|"mm_dve"|"full") strips pipeline stages for perf
    diagnosis (outputs are garbage except in "full")."""
    nc = bacc.Bacc("TRN2", target_bir_lowering=False, debug=False,
                   num_devices=N_CORES)
    f32 = mybir.dt.float32

    # Per-core external inputs (host-prepped layouts).
    bf16d = mybir.dt.bfloat16
    tp2 = nc.dram_tensor("t_part2", [2, N_SLICES * P], bf16d, kind="ExternalInput")
    op2 = nc.dram_tensor("o_part2", [2, N_SLICES * P], bf16d, kind="ExternalInput")
    gind = nc.dram_tensor("g_ind", [GPC, P], bf16d, kind="ExternalInput")
    rhs_t_d = nc.dram_tensor("rhs_t", [128, 512], bf16d, kind="ExternalInput")
    rhs_o_d = nc.dram_tensor("rhs_o", [128, 512], bf16d, kind="ExternalInput")

    m_acc_d = nc.dram_tensor("m_acc", [P, N_SLICES], f32, kind="ExternalOutput")

    with ExitStack() as ctx:
        tc = ctx.enter_context(tile.TileContext(nc, num_cores=N_CORES))
        consts = ctx.enter_context(tc.tile_pool(name="consts", bufs=1))
        psum_u = ctx.enter_context(tc.tile_pool(name="pu", bufs=3, space="PSUM"))
        psum_do = ctx.enter_context(tc.tile_pool(name="pdo", bufs=3, space="PSUM"))
        h_pool = ctx.enter_context(tc.tile_pool(name="hp", bufs=4))
        m_pool = ctx.enter_context(tc.tile_pool(name="mp", bufs=4))

        bf16 = mybir.dt.bfloat16
        KPAD = 128  # FWL needs a full 128-row stationary
        w_t = consts.tile([KPAD, N_SLICES * P], bf16)
        w_o = consts.tile([KPAD, N_SLICES * P], bf16)
        rhs_t = consts.tile([KPAD, 512], bf16)
        rhs_o = consts.tile([KPAD, 512], bf16)
        g_small = consts.tile([GPC, P], bf16)
        m_acc = consts.tile([P, N_SLICES], f32)

        nc.sync.dma_start(g_small[:], gind[:])
        nc.sync.dma_start(rhs_t[:], rhs_t_d[:])
        nc.sync.dma_start(rhs_o[:], rhs_o_d[:])
        # zero the FWL pad rows first; the value-row DMAs then overwrite 64-65
        nc.vector.memset(w_t[64:KPAD, :], 0.0)
        nc.scalar.memzero(w_o[64:KPAD, :])
        nc.sync.dma_start(w_t[GPC:KDIM, :], tp2[:])
        nc.sync.dma_start(w_o[GPC:KDIM, :], op2[:])

        # Replicate the fixed group-indicator block across all 64 slice
        # positions of each weight buffer (stride-0 read AP), split in half so
        # early matmuls only wait on the first piece.
        HREP = N_SLICES // 2
        for hh in range(2):
            src = g_small[:, None, :].broadcast_to((GPC, HREP, P))
            dst_t = w_t[0:GPC, hh * HREP * P:(hh + 1) * HREP * P]
            dst_o = w_o[0:GPC, hh * HREP * P:(hh + 1) * HREP * P]
            nc.vector.tensor_copy(dst_t.rearrange("g (r p) -> g r p", p=P), src)
            nc.scalar.copy(dst_o.rearrange("g (r p) -> g r p", p=P), src)

        loop_cm = tc.For_i(0, loop, 1) if loop else None
        if loop_cm is not None:
            loop_cm.__enter__()
        for s0 in range(N_SLICES * repeat):
            s = s0 % N_SLICES
            u_t = psum_u.tile([P, 512], f32, tag="u")
            do_t = psum_do.tile([P, 512], f32, tag="do")
            nc.tensor.matmul(
                u_t[:, 0:EXT[s]],
                lhsT=w_t[0:KPAD, s * P:(s + 1) * P],
                rhs=rhs_t[:, 0:EXT[s]],
                start=True, stop=True,
            )
            nc.tensor.matmul(
                do_t[:, 0:EXT[s]],
                lhsT=w_o[0:KPAD, s * P:(s + 1) * P],
                rhs=rhs_o[:, 0:EXT[s]],
                start=True, stop=True,
            )
            if mode == "mm":
                continue
            if mode == "mm_dve":
                m_t = m_pool.tile([P, 512], f32, tag="m")
                nc.vector.tensor_scalar(
                    out=m_t[:, 0:EXT[s]], in0=u_t[:, 0:EXT[s]], scalar1=0.0,
                    scalar2=1.0,
                    op0=mybir.AluOpType.is_gt, op1=mybir.AluOpType.mult,
                    accum_out=m_acc[:, s:s + 1],
                )
                continue
            h_t = h_pool.tile([P, 512], f32, tag="h")
            nc.scalar.activation(
                h_t[:, 0:EXT[s]], do_t[:, 0:EXT[s]],
                mybir.ActivationFunctionType.Relu,
            )
            if mode == "mm_act":
                continue
            m_t = m_pool.tile([P, 512], f32, tag="m")
            nc.vector.scalar_tensor_tensor(
                out=m_t[:, 0:EXT[s]], in0=u_t[:, 0:EXT[s]], scalar=0.0,
                in1=h_t[:, 0:EXT[s]],
                op0=mybir.AluOpType.is_gt, op1=mybir.AluOpType.mult,
                accum_out=m_acc[:, s:s + 1],
            )

        if loop_cm is not None:
            loop_cm.__exit__(None, None, None)
        if mode in ("full", "mm_dve"):
            nc.sync.dma_start(m_acc_d[:], m_acc[:])

    nc.compile()
    return nc


def _prep_core_inputs(t_groups, o_groups):
    """Build per-core input arrays from [GPC, D] group slabs (float32)."""
    tp = t_groups - np.float32(0.1)   # fold the -0.1 threshold into t_i
    op = o_groups + np.float32(1.0)   # fold the +1 hinge margin into o_i

    # [128, 128]: row p=(g_local*2+half) holds i_local values of that half.
    t_part = tp.reshape(GPC * 2, IPB)
    o_part = op.reshape(GPC * 2, IPB)
    # [2, N_SLICES*P]: row k, col s*P+p = value at (p, i_local=2s+k).
    tp2 = np.ascontiguousarray(
        t_part.T.reshape(N_SLICES, 2, P).transpose(1, 0, 2).reshape(2, N_SLICES * P))
    op2 = np.ascontiguousarray(
        o_part.T.reshape(N_SLICES, 2, P).transpose(1, 0, 2).reshape(2, N_SLICES * P))

    pidx = np.arange(P)
    gind = (pidx[None, :] // 2 == np.arange(GPC)[:, None]).astype(np.float32)

    # Interleaved (j, i') free layout: column j*2 + i' belongs to i-slot i',
    # decoy j. Each slice's live region is then the contiguous prefix
    # [0 : 2*JMAX), which the float32r moving-operand path requires.
    rhs_t = np.zeros((KDIM, 512), dtype=np.float32)
    rhs_o = np.zeros((KDIM, 512), dtype=np.float32)
    rhs_t[:GPC, 0::2] = -t_groups
    rhs_t[:GPC, 1::2] = -t_groups
    rhs_o[:GPC, 0::2] = -o_groups
    rhs_o[:GPC, 1::2] = -o_groups
    rhs_t[GPC, 0::2] = 1.0
    rhs_t[GPC + 1, 1::2] = 1.0
    rhs_o[GPC, 0::2] = 1.0
    rhs_o[GPC + 1, 1::2] = 1.0

    import ml_dtypes
    bf = ml_dtypes.bfloat16
    rhs_t_pad = np.zeros((128, 512), dtype=bf)
    rhs_o_pad = np.zeros((128, 512), dtype=bf)
    rhs_t_pad[:KDIM] = rhs_t.astype(bf)
    rhs_o_pad[:KDIM] = rhs_o.astype(bf)
    return {
        "t_part2": tp2.astype(bf), "o_part2": op2.astype(bf),
        "g_ind": gind.astype(bf),
        "rhs_t": rhs_t_pad, "rhs_o": rhs_o_pad,
    }


def kernel(input, gdt_ts):
    input = np.asarray(input)
    gdt_ts = np.asarray(gdt_ts)
    o = input.reshape(-1)[: G_REAL * D].astype(np.float32, copy=False)
    t = gdt_ts.reshape(-1)[: G_REAL * D].astype(np.float32, copy=False)

    t_g = np.zeros((G_PAD, D), dtype=np.float32)
    o_g = np.zeros((G_PAD, D), dtype=np.float32)
    t_g[:G_REAL] = t.reshape(G_REAL, D)
    o_g[:G_REAL] = o.reshape(G_REAL, D)
    # Sort decoys within each group by t (loss-invariant permutation); the
    # device program's shortened j-extents rely on active pairs having j < i.
    idx = np.argsort(t_g, axis=1)
    t_g = np.take_along_axis(t_g, idx, axis=1)
    o_g = np.take_along_axis(o_g, idx, axis=1)

    if "nc" not in _CACHED:
        _CACHED["nc"] = _build_program()
    nc = _CACHED["nc"]

    in_maps = []
    for c in range(N_CORES):
        sl = slice(c * GPC, (c + 1) * GPC)
        in_maps.append(_prep_core_inputs(t_g[sl], o_g[sl]))

    res = run_bass_kernel_spmd(nc, in_maps, list(range(N_CORES)))

    total = np.float64(0.0)
    for c in range(N_CORES):
        total += res.results[c]["m_acc"].astype(np.float64).sum()

    loss = 2.0 * total / float(N_PAIRS)
    return np.array([loss], dtype=np.float32)



# revision 3
# speedup vs baseline: 1347.5711x; 1347.5711x over previous
"""Trainium2 Bass kernel for BatchRankingLoss — sorted-prefix single-matmul design.

Reference (B=131072, d=256, K=512 complexes, G=511 groups):
    dt = t_i - t_j ; w = |dt| > 0.1 ; y = sign-ish(dt)
    dL = w * max(0, 1 + y*(o_i - o_j)) ; loss = sum(dL) / (G*d*(d-1))

Identity: dL is symmetric in (i,j) among active pairs, so
    sum(dL) = 2 * sum_{(i,j): t_j < t_i - 0.1} relu((1 + o_i) - o_j)

Host sorts each group's decoys by t, so the active j's for row i are exactly
the prefix j < c_i (c_i = #{j: t_j < t_i - 0.1}, non-decreasing in i).

Device layout per core (64 groups):
  partition p = (g_local = p//2, parity = p%2)
  slice s in [0,32): islot k in [0,4) covers decoy i = 8s + 2k + parity
  free axis interleaved: col = 4j + k, live prefix [0, 4*J_s),
  J_s = data-tight max c over the slice's rows (global over cores).

  One bf16 matmul per slice (contraction 68 = 64 group one-hots + 4 islot
  a-rows, both shipped from host inside W) produces do = (1+o_i) - o_j in
  fp32 PSUM; slices pack into [128, 2048] 4-bank PSUM tiles; each filled
  tile gets ONE relu+accumulate pass (ScalarE activation Relu+accum_out, or
  VectorE tensor_scalar max/add accum_out), tiles alternating between the
  two engines by a greedy time balance. Separate per-engine accum tiles
  keep ACT and DVE free of cross-engine tile hazards.

Pairs j in [c_i, J_s) are over-included by the shared extent; all are
inactive (|dt| <= 0.1) and their relu mass is subtracted exactly on the
host. Padded rows (group 511) use a = -1000 so they contribute exactly 0.
"""

import numpy as np
from contextlib import ExitStack

import concourse.bacc as bacc
import concourse.mybir as mybir
import concourse.tile as tile
from concourse.bass_utils import run_bass_kernel_spmd

import ml_dtypes

N_CORES = 8
D = 256
G_REAL = 511
G_PAD = 512
GPC = G_PAD // N_CORES   # 64 groups per core
P = 128                  # partitions: p = 2*g_local + parity
M = 4                    # islots per slice
WIN = 2 * M              # i-window per slice
N_SLICES = D // WIN      # 32
KDIM = GPC + M           # matmul contraction rows
RHS_W = M * D            # rhs moving-operand width
N_PAIRS = G_REAL * D * (D - 1)
PSUM_COLS = 1024         # fp32 cols per PSUM tile (2 banks)
PSUM_BUFS = 4
BANK = 512               # fp32 cols per PSUM bank (matmul piece limit)

THRESHOLD = np.float32(0.1)
PAD_A = np.float32(-1000.0)

_CACHED = {}


def _schedule(J):
    """Pack slices (extents E=4*J) into PSUM tiles. [(fill, [(s,off,E)])]"""
    tiles = []
    cur = []
    fill = 0
    for s in range(N_SLICES):
        E = M * int(J[s])
        if E == 0:
            continue
        if fill + E > PSUM_COLS:
            tiles.append((fill, cur))
            cur, fill = [], 0
        cur.append((s, fill, E))
        fill += E
    if cur:
        tiles.append((fill, cur))
    return tiles


def _assign_engines(tiles):
    """Greedy: give each tile to the engine with less accumulated time.
    Constants are sim-fit: ACT ~330ns/op + 1.2G/s, DVE ~130ns/op + 0.96G/s."""
    t_dve = t_act = 0.0
    out = []
    for fill, _ in tiles:
        dve_cost = 125.0 + fill / 0.96
        act_cost = 293.0 + fill / 1.2
        if t_dve + dve_cost <= t_act + act_cost:
            out.append("dve")
            t_dve += dve_cost
        else:
            out.append("act")
            t_act += act_cost
    return out


def _build_program(J, repeat=1, mode="full"):
    J = tuple(int(x) for x in J)
    tiles = _schedule(J)
    engines = _assign_engines(tiles)
    n_act = max(sum(1 for e in engines if e == "act"), 1)
    n_dve = max(sum(1 for e in engines if e == "dve"), 1)

    nc = bacc.Bacc("TRN2", target_bir_lowering=False, debug=False,
                   num_devices=N_CORES)
    f32 = mybir.dt.float32
    bf16 = mybir.dt.bfloat16

    W_CHUNKS = 4
    CS = N_SLICES // W_CHUNKS

    w_ds = [nc.dram_tensor(f"w{ch}", [KDIM, CS * P], bf16,
                           kind="ExternalInput") for ch in range(W_CHUNKS)]
    rhs_d = nc.dram_tensor("rhs", [KDIM, RHS_W], bf16, kind="ExternalInput")
    acc_a_d = nc.dram_tensor("acc_a", [P, n_act], f32, kind="ExternalOutput")
    acc_d_d = nc.dram_tensor("acc_d", [P, n_dve], f32, kind="ExternalOutput")

    with ExitStack() as ctx:
        tc = ctx.enter_context(tile.TileContext(nc, num_cores=N_CORES))
        consts = ctx.enter_context(tc.tile_pool(name="consts", bufs=1))
        psum = ctx.enter_context(
            tc.tile_pool(name="ps", bufs=PSUM_BUFS, space="PSUM"))
        h_dve = ctx.enter_context(tc.tile_pool(name="hd", bufs=2))
        h_act = ctx.enter_context(tc.tile_pool(name="ha", bufs=2))

        w_c = []
        for ch in range(W_CHUNKS):
            w_ci = consts.tile([KDIM, CS * P], bf16, name=f"w_c{ch}")
            w_c.append(w_ci)
        rhs_t = consts.tile([KDIM, RHS_W], bf16)
        acc_a = consts.tile([P, n_act], f32)
        acc_d = consts.tile([P, n_dve], f32)

        # Input DMAs: rhs on the gpsimd SWDGE queue (starts at t~100), w
        # chunks interleaved over the SP and ACT queues in need-order (the
        # ACT queue first pays the activation-table load).
        nc.gpsimd.dma_start(rhs_t[:], rhs_d[:])
        for ch in range(W_CHUNKS):
            eng = nc.sync if ch % 2 == 0 else nc.scalar
            eng.dma_start(w_c[ch][:], w_ds[ch][:])

        def lhsT_of(s):
            return w_c[s // CS][:, (s % CS) * P:(s % CS + 1) * P]

        ia = idv = 0
        for rep in range(repeat):
            ia = idv = 0
            for (fill, slices), eng in zip(tiles, engines):
                ps = psum.tile([P, PSUM_COLS], f32, tag="ps")
                for (s, off, E) in slices:
                    a = off
                    while a < off + E:
                        b = min((a // BANK + 1) * BANK, off + E)
                        nc.tensor.matmul(
                            ps[:, a:b],
                            lhsT=lhsT_of(s),
                            rhs=rhs_t[:, a - off:b - off],
                            start=True, stop=True,
                        )
                        a = b
                if mode == "mm":
                    continue
                if eng == "dve":
                    h = h_dve.tile([P, PSUM_COLS], f32, tag="hd")
                    nc.vector.tensor_scalar(
                        out=h[:, 0:fill], in0=ps[:, 0:fill],
                        scalar1=0.0, scalar2=None,
                        op0=mybir.AluOpType.max,
                        op1=mybir.AluOpType.add,
                        accum_out=acc_d[:, idv:idv + 1],
                    )
                    idv += 1
                else:
                    h = h_act.tile([P, PSUM_COLS], f32, tag="ha")
                    nc.scalar.activation(
                        h[:, 0:fill], ps[:, 0:fill],
                        mybir.ActivationFunctionType.Relu,
                        accum_out=acc_a[:, ia:ia + 1],
                    )
                    ia += 1

        if mode == "full":
            # separate queues so the two result tails overlap
            nc.scalar.dma_start(acc_a_d[:], acc_a[:])
            nc.sync.dma_start(acc_d_d[:], acc_d[:])

    nc.compile()
    return nc, tiles


def _host_prep(input, gdt_ts):
    """Sort groups by t, compute prefix counts c, extents J, per-core input
    arrays, and the exact over-inclusion correction."""
    o = np.asarray(input).reshape(-1)[:G_REAL * D].astype(np.float32)
    t = np.asarray(gdt_ts).reshape(-1)[:G_REAL * D].astype(np.float32)

    t_g = np.zeros((G_PAD, D), np.float32)
    o_g = np.zeros((G_PAD, D), np.float32)
    t_g[:G_REAL] = t.reshape(G_REAL, D)
    o_g[:G_REAL] = o.reshape(G_REAL, D)

    idx = np.argsort(t_g, axis=1, kind="stable")
    t_g = np.take_along_axis(t_g, idx, axis=1)
    o_g = np.take_along_axis(o_g, idx, axis=1)

    # c[g, i] = #{j : t_gj < t_gi - 0.1}; rows are sorted so this is a prefix.
    c = np.empty((G_PAD, D), np.int64)
    for g in range(G_PAD):
        c[g] = np.searchsorted(t_g[g], t_g[g] - THRESHOLD, side="left")

    a_g = (1.0 + o_g).astype(np.float32)
    a_g[G_REAL:] = PAD_A

    cw = c.reshape(G_PAD, N_SLICES, WIN)
    J = cw.max(axis=(0, 2))
    J = np.minimum((J + 3) // 4 * 4, D).astype(np.int64)

    # ---- exact over-inclusion correction ----
    Jrow = np.broadcast_to(J[:, None], (N_SLICES, WIN)).reshape(D)
    cr = c.copy()
    cr[G_REAL:] = D  # pad rows contribute exactly 0 — skip
    width = int((Jrow[None, :] - cr).max())
    corr = np.float64(0.0)
    for w in range(max(width, 0)):
        j = cr + w
        live = j < Jrow[None, :]
        jj = np.minimum(j, D - 1)
        v = a_g - o_g[np.arange(G_PAD)[:, None], jj]
        corr += np.where(live, np.maximum(v, 0.0), 0.0).sum(dtype=np.float64)

    # ---- per-core device arrays ----
    bf = ml_dtypes.bfloat16
    s_idx = np.arange(N_SLICES)
    k_idx = np.arange(M)
    p_idx = np.arange(P)
    i_map = (WIN * s_idx[:, None, None] + 2 * k_idx[None, :, None]
             + (p_idx % 2)[None, None, :])          # [S, M, P]
    gind = (p_idx[None, :] // 2 == np.arange(GPC)[:, None])  # [GPC, P]

    in_maps = []
    for cidx in range(N_CORES):
        gsl = slice(cidx * GPC, (cidx + 1) * GPC)
        a_core = a_g[gsl]
        o_core = o_g[gsl]
        g_of_p = p_idx // 2
        a_rows = a_core[g_of_p[None, None, :], i_map]    # [S, M, P]
        a_rows = a_rows.transpose(1, 0, 2).reshape(M, N_SLICES * P)

        rhs = np.zeros((KDIM, RHS_W), np.float32)
        rhs[:GPC].reshape(GPC, D, M)[:] = -o_core[:, :, None]
        for k in range(M):
            rhs[GPC + k, k::M] = 1.0

        w_full = np.zeros((KDIM, N_SLICES * P), np.float32)
        w_full[:GPC] = np.tile(gind, (1, N_SLICES))
        w_full[GPC:] = a_rows
        wbf = w_full.astype(bf)

        im = {"rhs": rhs.astype(bf)}
        W_CHUNKS = 4
        CS = N_SLICES // W_CHUNKS
        for ch in range(W_CHUNKS):
            im[f"w{ch}"] = np.ascontiguousarray(
                wbf[:, ch * CS * P:(ch + 1) * CS * P])
        in_maps.append(im)

    return in_maps, J, corr


def kernel(input, gdt_ts):
    in_maps, J, corr = _host_prep(input, gdt_ts)

    key = tuple(int(x) for x in J)
    if key not in _CACHED:
        _CACHED[key] = _build_program(J)
    nc, tiles = _CACHED[key]

    res = run_bass_kernel_spmd(nc, in_maps, list(range(N_CORES)))

    total = np.float64(0.0)
    for cidx in range(N_CORES):
        total += res.results[cidx]["acc_a"].astype(np.float64).sum()
        total += res.results[cidx]["acc_d"].astype(np.float64).sum()

    loss = 2.0 * (total - corr) / float(N_PAIRS)
    return np.array([loss], dtype=np.float32)


# revision 14
# speedup vs baseline: 1446.5813x; 1.0735x over previous
"""Trainium2 Bass kernel for BatchRankingLoss — sorted-prefix single-matmul design.

Reference (B=131072, d=256, K=512 complexes, G=511 groups):
    dt = t_i - t_j ; w = |dt| > 0.1 ; y = sign-ish(dt)
    dL = w * max(0, 1 + y*(o_i - o_j)) ; loss = sum(dL) / (G*d*(d-1))

Identity: dL is symmetric in (i,j) among active pairs, so
    sum(dL) = 2 * sum_{(i,j): t_j < t_i - 0.1} relu((1 + o_i) - o_j)

The host sorts each group's decoys by t, so the active j's for row i are
exactly the prefix j < c_i (c_i = #{j: t_j < t_i - 0.1}, non-decreasing
in i). This removes both the threshold-mask computation and the u-matmul
entirely: the device only evaluates relu((1+o_i) - o_j) over data-tight
prefix extents, and the handful of over-included boundary pairs (all
inactive, |dt| <= 0.1) are subtracted exactly on the host.

Device layout per core (64 groups):
  partition p = (g_local = p//2, parity = p%2)
  slice s in [0,32): islot k in [0,4) covers decoy i = 8s + 2k + parity
  free axis interleaved: col = 4j + k, live prefix [0, 4*J_s),
  J_s = data-tight max c over the slice's rows (global over cores,
  rounded up to 2; compiled program cached per J-schedule).

  One bf16 matmul per slice (contraction 68 = 64 group one-hot rows +
  4 islot a-rows, a = 1 + o_i) produces do = (1+o_i) - o_j in fp32 PSUM.
  Slice extents are packed column-continuously into [128, 1024] 2-bank
  PSUM tiles (bufs=4) so every tile fills completely; each tile gets ONE
  relu+accumulate pass — ScalarE activation(Relu, accum_out) or VectorE
  tensor_scalar(max, 0, accum-add) — assigned by a greedy time balance
  (ACT ~= 373ns + fill/1.2GHz, DVE ~= 125ns + fill/0.96GHz). ScalarE and
  VectorE drain different PSUM tiles concurrently; PE fills two tiles
  ahead. Weights stream in chunks across the SP and SWDGE DMA queues in
  first-use order; all-but-last accumulator columns DMA out early so only
  one column's latency rides the tail.

Padded rows (group 511) use a = -1000 so they contribute exactly 0.
"""

import numpy as np
from contextlib import ExitStack

import concourse.bacc as bacc
import concourse.mybir as mybir
import concourse.tile as tile
from concourse.bass_utils import run_bass_kernel_spmd

import ml_dtypes

N_CORES = 8
D = 256
G_REAL = 511
G_PAD = 512
GPC = G_PAD // N_CORES   # 64 groups per core
P = 128                  # partitions: p = 2*g_local + parity
M = 4                    # islots per slice
WIN = 2 * M              # i-window per slice
N_SLICES = D // WIN      # 32
KDIM = GPC + M           # matmul contraction rows
RHS_W = M * D            # rhs moving-operand width
N_PAIRS = G_REAL * D * (D - 1)
PSUM_COLS = 1024         # fp32 cols per PSUM tile (2 banks)
PSUM_BUFS = 4
BANK = 512               # fp32 cols per PSUM bank (matmul piece limit)

THRESHOLD = np.float32(0.1)
PAD_A = np.float32(-1000.0)

_CACHED = {}


def _schedule(J):
    """Column-continuous packing: slices may straddle PSUM-tile boundaries
    so every tile fills to PSUM_COLS. Returns [(fill, [(s, off, lo, hi)])]
    where the piece covers rhs cols [lo, hi) of slice s at tile offset off."""
    tiles = []
    cur = []
    fill = 0
    for s in range(N_SLICES):
        E = M * int(J[s])
        lo = 0
        while lo < E:
            take = min(E - lo, PSUM_COLS - fill)
            cur.append((s, fill, lo, lo + take))
            fill += take
            lo += take
            if fill == PSUM_COLS:
                tiles.append((fill, cur))
                cur, fill = [], 0
    if cur:
        tiles.append((fill, cur))
    return tiles


def _assign_engines(tiles):
    """Greedy: give each tile to the engine with less accumulated time.
    Constants are sim-fit: ACT ~330ns/op + 1.2G/s, DVE ~130ns/op + 0.96G/s."""
    t_dve = t_act = 0.0
    out = []
    for fill, _ in tiles:
        dve_cost = 125.0 + fill / 0.959
        act_cost = 373.0 + fill / 1.202
        if t_dve + dve_cost <= t_act + act_cost:
            out.append("dve")
            t_dve += dve_cost
        else:
            out.append("act")
            t_act += act_cost
    return out


def _build_program(J, repeat=1, mode="full"):
    J = tuple(int(x) for x in J)
    tiles = _schedule(J)
    engines = _assign_engines(tiles)
    n_act = max(sum(1 for e in engines if e == "act"), 1)
    n_dve = max(sum(1 for e in engines if e == "dve"), 1)

    assert sum(1 for e in engines if e == "act") >= 2
    assert sum(1 for e in engines if e == "dve") >= 2
    nc = bacc.Bacc("TRN2", target_bir_lowering=False, debug=False,
                   num_devices=N_CORES)
    f32 = mybir.dt.float32
    bf16 = mybir.dt.bfloat16

    active = [s for s in range(N_SLICES) if J[s] > 0]
    CS = 8
    chunks = [active[i:i + CS] for i in range(0, len(active), CS)]
    chunk_of = {}
    for ch, sl in enumerate(chunks):
        for pos, s in enumerate(sl):
            chunk_of[s] = (ch, pos)

    w_ds = [nc.dram_tensor(f"w{ch}", [KDIM, len(sl) * P], bf16,
                           kind="ExternalInput")
            for ch, sl in enumerate(chunks)]
    rhs_d = nc.dram_tensor("rhs", [KDIM, RHS_W], bf16, kind="ExternalInput")
    acc_a_d = nc.dram_tensor("acc_a", [P, n_act - 1], f32,
                             kind="ExternalOutput")
    acc_d_d = nc.dram_tensor("acc_d", [P, n_dve - 1], f32,
                             kind="ExternalOutput")
    accl_a_d = nc.dram_tensor("accl_a", [P, 1], f32, kind="ExternalOutput")
    accl_d_d = nc.dram_tensor("accl_d", [P, 1], f32, kind="ExternalOutput")

    with ExitStack() as ctx:
        tc = ctx.enter_context(tile.TileContext(nc, num_cores=N_CORES))
        consts = ctx.enter_context(tc.tile_pool(name="consts", bufs=1))
        psum = ctx.enter_context(
            tc.tile_pool(name="ps", bufs=PSUM_BUFS, space="PSUM"))
        h_dve = ctx.enter_context(tc.tile_pool(name="hd", bufs=2))
        h_act = ctx.enter_context(tc.tile_pool(name="ha", bufs=2))

        w_c = []
        for ch, sl in enumerate(chunks):
            w_ci = consts.tile([KDIM, len(sl) * P], bf16, name=f"w_c{ch}")
            w_c.append(w_ci)
        rhs_t = consts.tile([KDIM, RHS_W], bf16)
        acc_a = consts.tile([P, n_act - 1], f32)
        acc_d = consts.tile([P, n_dve - 1], f32)
        accl_a = consts.tile([P, 1], f32)
        accl_d = consts.tile([P, 1], f32)

        # Input DMAs: rhs + odd w chunks on the gpsimd SWDGE queue (starts
        # at t~100), even w chunks on SP. ACT's stream stays free so its
        # hoisted activation-table load is its only pre-pipeline work.
        nc.gpsimd.dma_start(rhs_t[:], rhs_d[:])
        for ch in range(len(chunks)):
            eng = nc.sync if ch % 2 == 0 else nc.gpsimd
            eng.dma_start(w_c[ch][:], w_ds[ch][:])

        def lhsT_of(s):
            ch, pos = chunk_of[s]
            return w_c[ch][:, pos * P:(pos + 1) * P]

        ia = idv = 0
        for rep in range(repeat):
            ia = idv = 0
            for (fill, slices), eng in zip(tiles, engines):
                ps = psum.tile([P, PSUM_COLS], f32, tag="ps")
                for (s, off, lo, hi) in slices:
                    a = off
                    while a < off + (hi - lo):
                        b = min((a // BANK + 1) * BANK, off + (hi - lo))
                        nc.tensor.matmul(
                            ps[:, a:b],
                            lhsT=lhsT_of(s),
                            rhs=rhs_t[:, lo + (a - off):lo + (b - off)],
                            start=True, stop=True,
                        )
                        a = b
                if mode == "mm":
                    continue
                if eng == "dve":
                    h = h_dve.tile([P, PSUM_COLS], f32, tag="hd")
                    last = idv == n_dve - 1
                    nc.vector.tensor_scalar(
                        out=h[:, 0:fill], in0=ps[:, 0:fill],
                        scalar1=0.0, scalar2=None,
                        op0=mybir.AluOpType.max,
                        op1=mybir.AluOpType.add,
                        accum_out=(accl_d[:, 0:1] if last
                                   else acc_d[:, idv:idv + 1]),
                    )
                    if idv == n_dve - 2 and mode == "full" and rep == repeat - 1:
                        nc.sync.dma_start(acc_d_d[:], acc_d[:])
                    idv += 1
                else:
                    h = h_act.tile([P, PSUM_COLS], f32, tag="ha")
                    last = ia == n_act - 1
                    nc.scalar.activation(
                        h[:, 0:fill], ps[:, 0:fill],
                        mybir.ActivationFunctionType.Relu,
                        accum_out=(accl_a[:, 0:1] if last
                                   else acc_a[:, ia:ia + 1]),
                    )
                    if ia == n_act - 2 and mode == "full" and rep == repeat - 1:
                        nc.sync.dma_start(acc_a_d[:], acc_a[:])
                    ia += 1

        if mode == "full":
            # only the last column of each engine rides the critical tail
            nc.scalar.dma_start(accl_a_d[:], accl_a[:])
            nc.sync.dma_start(accl_d_d[:], accl_d[:])

    nc.compile()
    return nc, tiles


def _host_prep(input, gdt_ts):
    """Sort groups by t, compute prefix counts c, extents J, per-core input
    arrays, and the exact over-inclusion correction."""
    o = np.asarray(input).reshape(-1)[:G_REAL * D].astype(np.float32)
    t = np.asarray(gdt_ts).reshape(-1)[:G_REAL * D].astype(np.float32)

    t_g = np.zeros((G_PAD, D), np.float32)
    o_g = np.zeros((G_PAD, D), np.float32)
    t_g[:G_REAL] = t.reshape(G_REAL, D)
    o_g[:G_REAL] = o.reshape(G_REAL, D)

    idx = np.argsort(t_g, axis=1, kind="stable")
    t_g = np.take_along_axis(t_g, idx, axis=1)
    o_g = np.take_along_axis(o_g, idx, axis=1)

    # c[g, i] = #{j : t_gj < t_gi - 0.1}; rows are sorted so this is a prefix.
    c = np.empty((G_PAD, D), np.int64)
    for g in range(G_PAD):
        c[g] = np.searchsorted(t_g[g], t_g[g] - THRESHOLD, side="left")

    a_g = (1.0 + o_g).astype(np.float32)
    a_g[G_REAL:] = PAD_A

    cw = c.reshape(G_PAD, N_SLICES, WIN)
    J = cw.max(axis=(0, 2))
    J = np.minimum((J + 1) // 2 * 2, D).astype(np.int64)

    # ---- exact over-inclusion correction ----
    Jrow = np.broadcast_to(J[:, None], (N_SLICES, WIN)).reshape(D)
    cr = c.copy()
    cr[G_REAL:] = D  # pad rows contribute exactly 0 — skip
    width = int((Jrow[None, :] - cr).max())
    corr = np.float64(0.0)
    for w in range(max(width, 0)):
        j = cr + w
        live = j < Jrow[None, :]
        jj = np.minimum(j, D - 1)
        v = a_g - o_g[np.arange(G_PAD)[:, None], jj]
        corr += np.where(live, np.maximum(v, 0.0), 0.0).sum(dtype=np.float64)

    # ---- per-core device arrays ----
    bf = ml_dtypes.bfloat16
    s_idx = np.arange(N_SLICES)
    k_idx = np.arange(M)
    p_idx = np.arange(P)
    i_map = (WIN * s_idx[:, None, None] + 2 * k_idx[None, :, None]
             + (p_idx % 2)[None, None, :])          # [S, M, P]
    gind = (p_idx[None, :] // 2 == np.arange(GPC)[:, None])  # [GPC, P]

    in_maps = []
    for cidx in range(N_CORES):
        gsl = slice(cidx * GPC, (cidx + 1) * GPC)
        a_core = a_g[gsl]
        o_core = o_g[gsl]
        g_of_p = p_idx // 2
        a_rows = a_core[g_of_p[None, None, :], i_map]    # [S, M, P]
        a_rows = a_rows.transpose(1, 0, 2).reshape(M, N_SLICES * P)

        rhs = np.zeros((KDIM, RHS_W), np.float32)
        rhs[:GPC].reshape(GPC, D, M)[:] = -o_core[:, :, None]
        for k in range(M):
            rhs[GPC + k, k::M] = 1.0

        w_full = np.zeros((KDIM, N_SLICES * P), np.float32)
        w_full[:GPC] = np.tile(gind, (1, N_SLICES))
        w_full[GPC:] = a_rows
        wbf = w_full.astype(bf)

        im = {"rhs": rhs.astype(bf)}
        active = [s for s in range(N_SLICES) if J[s] > 0]
        CS = 8
        chunks = [active[i:i + CS] for i in range(0, len(active), CS)]
        for ch, sl in enumerate(chunks):
            im[f"w{ch}"] = np.ascontiguousarray(
                np.concatenate([wbf[:, s * P:(s + 1) * P] for s in sl],
                               axis=1))
        in_maps.append(im)

    return in_maps, J, corr


def kernel(input, gdt_ts):
    in_maps, J, corr = _host_prep(input, gdt_ts)

    key = tuple(int(x) for x in J)
    if key not in _CACHED:
        _CACHED[key] = _build_program(J)
    nc, tiles = _CACHED[key]

    res = run_bass_kernel_spmd(nc, in_maps, list(range(N_CORES)))

    total = np.float64(0.0)
    for cidx in range(N_CORES):
        total += res.results[cidx]["acc_a"].astype(np.float64).sum()
        total += res.results[cidx]["acc_d"].astype(np.float64).sum()
        total += res.results[cidx]["accl_a"].astype(np.float64).sum()
        total += res.results[cidx]["accl_d"].astype(np.float64).sum()

    loss = 2.0 * (total - corr) / float(N_PAIRS)
    return np.array([loss], dtype=np.float32)
